# revision 21
# baseline (speedup 1.0000x reference)
"""DfOpOneStep (nn_DfOpOneStep_55765855371911) Trainium2 Bass kernel.

Contract: kernel(**inputs) takes FULL unsharded inputs
  spec_frame  [32768, 1, 1, 481, 2] f32
  coefs_frame [32768, 5, 1, 96, 2]  f32
  spec_buf    [32768, 1, 5, 96, 2]  f32
and returns the FULL output tuple
  (enhanced [32768, 1, 1, 481, 2], new_spec_buf [32768, 1, 5, 96, 2]).

Sharding: pure data-parallel over the batch axis across 8 NeuronCores
(4096 samples each). Per-core Bass program streams 512-sample tiles
(4 samples x 128 partitions):
  - high bins 96:481 of enhanced are a pure passthrough: moved with a
    single DRAM->DRAM DMA (one SDMA traversal instead of load+store,
    no SBUF round trip)
  - loads: spec bins 0:96, coef rows (contiguous), spec_buf taps 1..4
  - DVE: 4 complex-product pairs split over {taps 0-3, tap 4}, sub/add,
    then a 3-add tap-reduction tree (contiguous slices; beats the
    strided tensor_reduce ~2x) writing re/im interleaved into out_lo
  - stores: new_spec_buf taps 0-3 <- buf tile, tap 4 <- spec_lo tile,
    enhanced bins 0:96 <- out_lo
HBM traffic is the minimum possible (each input byte read once except
spec_buf tap 0 which is never read; each output byte written once).
"""

import sys

if "/opt/trn_rl_repo" not in sys.path:
    sys.path.insert(0, "/opt/trn_rl_repo")

import numpy as np

import concourse.bacc as bacc
import concourse.bass as bass
import concourse.tile as tile
from concourse import mybir
from concourse.bass_utils import run_bass_kernel_spmd

F32 = mybir.dt.float32
N_CORES = 8
BATCH = 32768
B_LOC = BATCH // N_CORES  # 4096
P = 128  # SBUF partitions
SPP = 4  # samples per partition per tile
O = 5  # DF order (taps)
F = 96  # DF-filtered bins
FW = 481  # full-band bins
ROW_S = 2 * FW  # 962 floats per spec row
ROW_C = O * F * 2  # 960 floats per coef/buf row
LO = 2 * F  # 192 floats of DF-filtered bins per row


def _build(b_loc: int = B_LOC, spp: int = SPP) -> bass.Bass:
    t_rows = P * spp
    assert b_loc % t_rows == 0
    n_iter = b_loc // t_rows

    # Bacc (not plain Bass): its compile() pass lowers the multi-sync-wait
    # instructions Tile emits into forms walrus can encode — plain Bass BIR
    # dies in walrus codegen with "Too many sync wait commands".
    nc = bacc.Bacc("TRN2", target_bir_lowering=False, debug=False, num_devices=N_CORES)
    spec = nc.declare_dram_parameter("spec_frame", [b_loc, ROW_S], F32, isOutput=False)
    coef = nc.declare_dram_parameter("coefs_frame", [b_loc, ROW_C], F32, isOutput=False)
    sbuf = nc.declare_dram_parameter("spec_buf", [b_loc, ROW_C], F32, isOutput=False)
    enh = nc.declare_dram_parameter("enhanced", [b_loc, ROW_S], F32, isOutput=True)
    nbuf = nc.declare_dram_parameter("new_spec_buf", [b_loc, ROW_C], F32, isOutput=True)

    with tile.TileContext(nc) as tc:
        with (
            tc.tile_pool(name="slo", bufs=4) as slo_pool,
            tc.tile_pool(name="coef", bufs=4) as coef_pool,
            tc.tile_pool(name="buf", bufs=4) as buf_pool,
            tc.tile_pool(name="prod", bufs=1) as prod_pool,
            tc.tile_pool(name="dsum", bufs=1) as d_pool,
            tc.tile_pool(name="pfold", bufs=1) as a_pool,
            tc.tile_pool(name="pcomb", bufs=1) as b_pool,
            tc.tile_pool(name="olo", bufs=3) as out_pool,
        ):
            # full tiles, except the last one split in half: the final DVE
            # chunk + dependent store sit serially after the last loads, so
            # a smaller last chunk shortens the kernel tail
            schedule = [(i * t_rows, spp) for i in range(n_iter - 1)]
            schedule.append(((n_iter - 1) * t_rows, spp // 2))
            schedule.append(((n_iter - 1) * t_rows + t_rows // 2, spp // 2))
            for r0, sppi in schedule:
                rows = slice(r0, r0 + P * sppi)

                slo_t = slo_pool.tile([P, sppi, F, 2], F32)
                coef_t = coef_pool.tile([P, sppi, O, F, 2], F32)
                buf_t = buf_pool.tile([P, sppi, O - 1, F, 2], F32)
                prod_t = prod_pool.tile([P, 4, sppi, O, F], F32)
                d_t = d_pool.tile([P, 2, sppi, O, F], F32)
                a_t = a_pool.tile([P, 2, sppi, 2, F], F32)
                b_t = b_pool.tile([P, 2, sppi, F], F32)
                out_t = out_pool.tile([P, sppi, F, 2], F32)

                # ---- passthrough high bins: one DRAM->DRAM DMA ----
                nc.scalar.dma_start(
                    out=enh[rows, LO:].rearrange("(p s) e -> p s e", p=P),
                    in_=spec[rows, LO:].rearrange("(p s) e -> p s e", p=P),
                )

                # ---- loads (HWDGE via sync ring); slo first — the nbuf tap-4
                # store on the ACT ring reads it, and a late slo load would
                # head-of-line block that whole ring ----
                nc.sync.dma_start(
                    out=slo_t[:],
                    in_=spec[rows, 0:LO].rearrange("(p s) (f c) -> p s f c", p=P, c=2),
                )
                nc.sync.dma_start(
                    out=coef_t[:],
                    in_=coef[rows, :].rearrange(
                        "(p s) (o f c) -> p s o f c", p=P, o=O, f=F
                    ),
                )
                nc.sync.dma_start(
                    out=buf_t[:],
                    in_=sbuf[rows, LO:].rearrange(
                        "(p s) (o f c) -> p s o f c", p=P, o=O - 1, f=F
                    ),
                )

                # ---- complex MAC on DVE ----
                # taps 0..3 of the new ring buffer = old spec_buf taps 1..4
                b_re = buf_t[:, :, :, :, 0]
                b_im = buf_t[:, :, :, :, 1]
                cl_re = coef_t[:, :, 0 : O - 1, :, 0]
                cl_im = coef_t[:, :, 0 : O - 1, :, 1]
                # tap 4 of the new ring buffer = current frame bins 0:96
                t_re = slo_t[:, :, :, 0]
                t_im = slo_t[:, :, :, 1]
                c4_re = coef_t[:, :, O - 1, :, 0]
                c4_im = coef_t[:, :, O - 1, :, 1]

                for k, (a, c) in enumerate(
                    [(b_re, cl_re), (b_im, cl_im), (b_re, cl_im), (b_im, cl_re)]
                ):
                    nc.vector.tensor_mul(prod_t[:, k, :, 0 : O - 1, :], a, c)
                for k, (a, c) in enumerate(
                    [(t_re, c4_re), (t_im, c4_im), (t_re, c4_im), (t_im, c4_re)]
                ):
                    nc.vector.tensor_mul(prod_t[:, k, :, O - 1, :], a, c)

                # NOTE: running these two on nc.gpsimd (to offload DVE) makes
                # the device die with NRT_EXEC_UNIT_UNRECOVERABLE — keep on DVE
                nc.vector.tensor_sub(d_t[:, 0], prod_t[:, 0], prod_t[:, 1])
                nc.vector.tensor_add(d_t[:, 1], prod_t[:, 2], prod_t[:, 3])

                # ---- stores with no compute deps (ACT HWDGE ring) ----
                nc.scalar.dma_start(
                    out=nbuf[rows, 0 : 2 * (O - 1) * F].rearrange(
                        "(p s) (o f c) -> p s o f c", p=P, o=O - 1, f=F
                    ),
                    in_=buf_t[:],
                )
                nc.scalar.dma_start(
                    out=nbuf[rows, 2 * (O - 1) * F :].rearrange(
                        "(p s) e -> p s e", p=P
                    ),
                    in_=slo_t[:].rearrange("p s f c -> p s (f c)"),
                )

                # ---- tap reduction as an add tree (contiguous slices beat
                # the strided tensor_reduce ~2x), re/im interleaved out ----
                for comp in range(2):
                    dc = d_t[:, comp]
                    nc.vector.tensor_add(
                        a_t[:, comp], dc[:, :, 0:2, :], dc[:, :, 2:4, :]
                    )
                    nc.vector.tensor_add(
                        b_t[:, comp], a_t[:, comp, :, 0, :], dc[:, :, 4, :]
                    )
                    nc.vector.tensor_add(
                        out_t[:, :, :, comp], b_t[:, comp], a_t[:, comp, :, 1, :]
                    )

                # ---- enhanced low bins store (depends on DVE tree) ----
                nc.scalar.dma_start(
                    out=enh[rows, 0:LO].rearrange("(p s) e -> p s e", p=P),
                    in_=out_t[:].rearrange("p s f c -> p s (f c)"),
                )
    nc.finalize()
    return nc


_NC_CACHE: bass.Bass | None = None


def _get_nc() -> bass.Bass:
    global _NC_CACHE
    if _NC_CACHE is None:
        _NC_CACHE = _build()
    return _NC_CACHE


def _run(spec_frame, coefs_frame, spec_buf, trace: bool = False, tmpdir=None):
    """Shard, execute on 8 cores, gather. Returns (enhanced, new_spec_buf, results)."""
    spec2 = np.ascontiguousarray(spec_frame, dtype=np.float32).reshape(BATCH, ROW_S)
    coef2 = np.ascontiguousarray(coefs_frame, dtype=np.float32).reshape(BATCH, ROW_C)
    sbuf2 = np.ascontiguousarray(spec_buf, dtype=np.float32).reshape(BATCH, ROW_C)

    in_maps = [
        {
            "spec_frame": spec2[c * B_LOC : (c + 1) * B_LOC],
            "coefs_frame": coef2[c * B_LOC : (c + 1) * B_LOC],
            "spec_buf": sbuf2[c * B_LOC : (c + 1) * B_LOC],
        }
        for c in range(N_CORES)
    ]
    res = run_bass_kernel_spmd(
        _get_nc(), in_maps, list(range(N_CORES)), trace=trace, tmpdir=tmpdir
    )

    enhanced = np.concatenate(
        [res.results[c]["enhanced"] for c in range(N_CORES)], axis=0
    ).reshape(BATCH, 1, 1, FW, 2)
    new_spec_buf = np.concatenate(
        [res.results[c]["new_spec_buf"] for c in range(N_CORES)], axis=0
    ).reshape(BATCH, 1, O, F, 2)
    return enhanced, new_spec_buf, res


def kernel(spec_frame, coefs_frame, spec_buf):
    enhanced, new_spec_buf, _ = _run(spec_frame, coefs_frame, spec_buf)
    return enhanced, new_spec_buf


# revision 22
# speedup vs baseline: 1.2489x; 1.2489x over previous
"""DfOpOneStep (nn_DfOpOneStep_55765855371911) Trainium2 Bass kernel.

Contract: kernel(**inputs) takes FULL unsharded inputs
  spec_frame  [32768, 1, 1, 481, 2] f32
  coefs_frame [32768, 5, 1, 96, 2]  f32
  spec_buf    [32768, 1, 5, 96, 2]  f32
and returns the FULL output tuple
  (enhanced [32768, 1, 1, 481, 2], new_spec_buf [32768, 1, 5, 96, 2]).

Sharding: pure data-parallel over the batch axis across 8 NeuronCores
(4096 samples each). Per-core Bass program streams 512-sample tiles
(4 samples x 128 partitions):
  - high bins 96:481 of enhanced are a pure passthrough: moved with a
    single DRAM->DRAM DMA (one SDMA traversal instead of load+store,
    no SBUF round trip)
  - loads: spec bins 0:96, coef rows (contiguous), spec_buf taps 1..4
  - DVE: 4 complex-product pairs split over {taps 0-3, tap 4}, sub/add,
    then a 3-add tap-reduction tree (contiguous slices; beats the
    strided tensor_reduce ~2x) writing re/im interleaved into out_lo
  - stores: new_spec_buf taps 0-3 <- buf tile, tap 4 <- spec_lo tile,
    enhanced bins 0:96 <- out_lo
HBM traffic is the minimum possible (each input byte read once except
spec_buf tap 0 which is never read; each output byte written once).
"""

import sys

if "/opt/trn_rl_repo" not in sys.path:
    sys.path.insert(0, "/opt/trn_rl_repo")

import numpy as np

import concourse.bacc as bacc
import concourse.bass as bass
import concourse.tile as tile
from concourse import mybir
from concourse.bass_utils import run_bass_kernel_spmd

F32 = mybir.dt.float32
N_CORES = 8
BATCH = 32768
B_LOC = BATCH // N_CORES  # 4096
P = 128  # SBUF partitions
SPP = 4  # samples per partition per tile
O = 5  # DF order (taps)
F = 96  # DF-filtered bins
FW = 481  # full-band bins
ROW_S = 2 * FW  # 962 floats per spec row
ROW_C = O * F * 2  # 960 floats per coef/buf row
LO = 2 * F  # 192 floats of DF-filtered bins per row


def _build(b_loc: int = B_LOC, spp: int = SPP) -> bass.Bass:
    t_rows = P * spp
    assert b_loc % t_rows == 0
    n_iter = b_loc // t_rows

    # Bacc (not plain Bass): its compile() pass lowers the multi-sync-wait
    # instructions Tile emits into forms walrus can encode — plain Bass BIR
    # dies in walrus codegen with "Too many sync wait commands".
    nc = bacc.Bacc("TRN2", target_bir_lowering=False, debug=False, num_devices=N_CORES)
    spec = nc.declare_dram_parameter("spec_frame", [b_loc, ROW_S], F32, isOutput=False)
    coef = nc.declare_dram_parameter("coefs_frame", [b_loc, ROW_C], F32, isOutput=False)
    sbuf = nc.declare_dram_parameter("spec_buf", [b_loc, ROW_C], F32, isOutput=False)
    enh = nc.declare_dram_parameter("enhanced", [b_loc, ROW_S], F32, isOutput=True)
    nbuf = nc.declare_dram_parameter("new_spec_buf", [b_loc, ROW_C], F32, isOutput=True)

    with tile.TileContext(nc) as tc:
        with (
            tc.tile_pool(name="slo", bufs=4) as slo_pool,
            tc.tile_pool(name="coef", bufs=4) as coef_pool,
            tc.tile_pool(name="buf", bufs=4) as buf_pool,
            tc.tile_pool(name="prod", bufs=1) as prod_pool,
            tc.tile_pool(name="dsum", bufs=1) as d_pool,
            tc.tile_pool(name="pfold", bufs=1) as a_pool,
            tc.tile_pool(name="pcomb", bufs=1) as b_pool,
            tc.tile_pool(name="olo", bufs=3) as out_pool,
        ):
            for i in range(n_iter):
                rows = slice(i * t_rows, (i + 1) * t_rows)

                slo_t = slo_pool.tile([P, spp, F, 2], F32)
                coef_t = coef_pool.tile([P, spp, O, F, 2], F32)
                buf_t = buf_pool.tile([P, spp, O - 1, F, 2], F32)
                prod_t = prod_pool.tile([P, 4, spp, O, F], F32)
                d_t = d_pool.tile([P, 2, spp, O, F], F32)
                a_t = a_pool.tile([P, 2, spp, 2, F], F32)
                b_t = b_pool.tile([P, 2, spp, F], F32)
                out_t = out_pool.tile([P, spp, F, 2], F32)

                # ---- passthrough high bins: one DRAM->DRAM DMA ----
                nc.scalar.dma_start(
                    out=enh[rows, LO:].rearrange("(p s) e -> p s e", p=P),
                    in_=spec[rows, LO:].rearrange("(p s) e -> p s e", p=P),
                )

                # ---- loads (HWDGE via sync ring); slo first — the nbuf tap-4
                # store on the ACT ring reads it, and a late slo load would
                # head-of-line block that whole ring ----
                nc.sync.dma_start(
                    out=slo_t[:],
                    in_=spec[rows, 0:LO].rearrange("(p s) (f c) -> p s f c", p=P, c=2),
                )
                nc.sync.dma_start(
                    out=coef_t[:],
                    in_=coef[rows, :].rearrange(
                        "(p s) (o f c) -> p s o f c", p=P, o=O, f=F
                    ),
                )
                nc.sync.dma_start(
                    out=buf_t[:],
                    in_=sbuf[rows, LO:].rearrange(
                        "(p s) (o f c) -> p s o f c", p=P, o=O - 1, f=F
                    ),
                )

                # ---- complex MAC on DVE ----
                # taps 0..3 of the new ring buffer = old spec_buf taps 1..4
                b_re = buf_t[:, :, :, :, 0]
                b_im = buf_t[:, :, :, :, 1]
                cl_re = coef_t[:, :, 0 : O - 1, :, 0]
                cl_im = coef_t[:, :, 0 : O - 1, :, 1]
                # tap 4 of the new ring buffer = current frame bins 0:96
                t_re = slo_t[:, :, :, 0]
                t_im = slo_t[:, :, :, 1]
                c4_re = coef_t[:, :, O - 1, :, 0]
                c4_im = coef_t[:, :, O - 1, :, 1]

                for k, (a, c) in enumerate(
                    [(b_re, cl_re), (b_im, cl_im), (b_re, cl_im), (b_im, cl_re)]
                ):
                    nc.vector.tensor_mul(prod_t[:, k, :, 0 : O - 1, :], a, c)
                for k, (a, c) in enumerate(
                    [(t_re, c4_re), (t_im, c4_im), (t_re, c4_im), (t_im, c4_re)]
                ):
                    nc.vector.tensor_mul(prod_t[:, k, :, O - 1, :], a, c)

                # NOTE: running these two on nc.gpsimd (to offload DVE) makes
                # the device die with NRT_EXEC_UNIT_UNRECOVERABLE — keep on DVE
                nc.vector.tensor_sub(d_t[:, 0], prod_t[:, 0], prod_t[:, 1])
                nc.vector.tensor_add(d_t[:, 1], prod_t[:, 2], prod_t[:, 3])

                # ---- stores with no compute deps (ACT HWDGE ring) ----
                nc.scalar.dma_start(
                    out=nbuf[rows, 0 : 2 * (O - 1) * F].rearrange(
                        "(p s) (o f c) -> p s o f c", p=P, o=O - 1, f=F
                    ),
                    in_=buf_t[:],
                )
                nc.scalar.dma_start(
                    out=nbuf[rows, 2 * (O - 1) * F :].rearrange(
                        "(p s) e -> p s e", p=P
                    ),
                    in_=slo_t[:].rearrange("p s f c -> p s (f c)"),
                )

                # ---- tap reduction as an add tree (contiguous slices beat
                # the strided tensor_reduce ~2x), re/im interleaved out ----
                for comp in range(2):
                    dc = d_t[:, comp]
                    nc.vector.tensor_add(
                        a_t[:, comp], dc[:, :, 0:2, :], dc[:, :, 2:4, :]
                    )
                    nc.vector.tensor_add(
                        b_t[:, comp], a_t[:, comp, :, 0, :], dc[:, :, 4, :]
                    )
                    nc.vector.tensor_add(
                        out_t[:, :, :, comp], b_t[:, comp], a_t[:, comp, :, 1, :]
                    )

                # ---- enhanced low bins store (depends on DVE tree) ----
                nc.scalar.dma_start(
                    out=enh[rows, 0:LO].rearrange("(p s) e -> p s e", p=P),
                    in_=out_t[:].rearrange("p s f c -> p s (f c)"),
                )
    nc.finalize()
    return nc


_NC_CACHE: bass.Bass | None = None


def _get_nc() -> bass.Bass:
    global _NC_CACHE
    if _NC_CACHE is None:
        _NC_CACHE = _build()
    return _NC_CACHE


def _run(spec_frame, coefs_frame, spec_buf, trace: bool = False, tmpdir=None):
    """Shard, execute on 8 cores, gather. Returns (enhanced, new_spec_buf, results)."""
    spec2 = np.ascontiguousarray(spec_frame, dtype=np.float32).reshape(BATCH, ROW_S)
    coef2 = np.ascontiguousarray(coefs_frame, dtype=np.float32).reshape(BATCH, ROW_C)
    sbuf2 = np.ascontiguousarray(spec_buf, dtype=np.float32).reshape(BATCH, ROW_C)

    in_maps = [
        {
            "spec_frame": spec2[c * B_LOC : (c + 1) * B_LOC],
            "coefs_frame": coef2[c * B_LOC : (c + 1) * B_LOC],
            "spec_buf": sbuf2[c * B_LOC : (c + 1) * B_LOC],
        }
        for c in range(N_CORES)
    ]
    res = run_bass_kernel_spmd(
        _get_nc(), in_maps, list(range(N_CORES)), trace=trace, tmpdir=tmpdir
    )

    enhanced = np.concatenate(
        [res.results[c]["enhanced"] for c in range(N_CORES)], axis=0
    ).reshape(BATCH, 1, 1, FW, 2)
    new_spec_buf = np.concatenate(
        [res.results[c]["new_spec_buf"] for c in range(N_CORES)], axis=0
    ).reshape(BATCH, 1, O, F, 2)
    return enhanced, new_spec_buf, res


def kernel(spec_frame, coefs_frame, spec_buf):
    enhanced, new_spec_buf, _ = _run(spec_frame, coefs_frame, spec_buf)
    return enhanced, new_spec_buf
